# revision 22
# baseline (speedup 1.0000x reference)
"""BitnetMLP on 8 TRN2 NeuronCores — Megatron tensor-parallel over the
intermediate dim I, exact integer arithmetic on the TensorEngine.

v2: all quantization that only needs host-visible data moves to the host:
  - weights are ternarized on host and shipped as fp8e4 {-1,0,+1},
  - x is int8-quantized on host; shipped as bf16 qx*2^e (exact), with the
    per-token residual r = (absmax/127)/2^e in [1,2) shipped as an f32 row.
This removes the on-device weight-stats pass + AllReduce + weight quant pass
and the per-group x-quant prepass entirely, so matmuls start immediately.

Device math per core r (I-shard of 1024):
  g_ps/u_ps = ternary x int8 matmuls (exact, f32 PSUM).
  h/r = silu(g_ps*r*mg)*(u_ps*mu)    (the token residual r folded into stats)
  per-token stats sum(h^2), max|lnw*h| -> AllGather (8 cores) -> rms scale +
  int8 requant scale; qh int8 -> down matmul -> dequant -> bf16 partial ->
  ReduceScatter(add) -> f32 output rows.
"""
import numpy as np
import ml_dtypes

N_CORES = 8
B, S, H, I = 2, 2048, 2048, 8192
T = B * S                      # 4096 tokens
ISH = I // N_CORES             # 1024  I shard per core
TG = 512                       # tokens per group
NG = T // TG                   # 8 groups
KC = H // 128                  # 16 contract chunks for gate/up
IC = ISH // 128                # 8  contract chunks for down / h^T partition chunks
NH = 2048 // 512               # 4  output col groups for down
NTC = TG // 128                # 4  token tiles per group

MAGIC = float(1.5 * 2 ** 23)   # f32 round-to-nearest-even forcing constant
EPS = 1e-5
RMS_EPS = 1e-6

_CACHED = {}


def _build():
    import concourse.bass as bass
    import concourse.bacc as bacc
    import concourse.tile as tile
    import concourse.mybir as mybir
    from concourse import masks
    from contextlib import ExitStack

    dt = mybir.dt
    AO = mybir.AluOpType
    AF = mybir.ActivationFunctionType
    RG = [list(range(N_CORES))]

    nc = bacc.Bacc("TRN2", target_bir_lowering=False, debug=False,
                   num_devices=N_CORES)

    qxT_in = nc.dram_tensor("qxT", [H, T], dt.bfloat16, kind="ExternalInput")
    wgT_in = nc.dram_tensor("wgT", [H, ISH], dt.float8e4, kind="ExternalInput")
    wuT_in = nc.dram_tensor("wuT", [H, ISH], dt.float8e4, kind="ExternalInput")
    wdT_in = nc.dram_tensor("wdT", [ISH, 2048], dt.float8e4,
                            kind="ExternalInput")
    lnw_in = nc.dram_tensor("lnw", [ISH], dt.float32, kind="ExternalInput")
    rrow_in = nc.dram_tensor("rrow", [T], dt.float32, kind="ExternalInput")
    scl_in = nc.dram_tensor("scl", [8], dt.float32, kind="ExternalInput")
    y_out = nc.dram_tensor("y_out", [T // N_CORES, 2048], dt.bfloat16,
                           kind="ExternalOutput")

    with tile.TileContext(nc) as tc:
        with ExitStack() as stack:
            ep = stack.enter_context
            constp = ep(tc.tile_pool(name="const", bufs=1))
            wqp = ep(tc.tile_pool(name="wq", bufs=1))
            qxp = ep(tc.tile_pool(name="qx", bufs=2))
            hbp = ep(tc.tile_pool(name="hbuf", bufs=2))
            qhp = ep(tc.tile_pool(name="qh", bufs=2))
            sxp = ep(tc.tile_pool(name="sxal", bufs=2))
            yrp = ep(tc.tile_pool(name="yrow", bufs=4))
            smp = ep(tc.tile_pool(name="small", bufs=2))
            rowp = ep(tc.tile_pool(name="rows", bufs=2))
            evp = ep(tc.tile_pool(name="evac", bufs=2))
            h2p = ep(tc.tile_pool(name="h2", bufs=10))
            ps_gu = ep(tc.tile_pool(name="ps_gu", bufs=4, space="PSUM"))
            ps_dn = ep(tc.tile_pool(name="ps_dn", bufs=2, space="PSUM"))
            ps_ss = ep(tc.tile_pool(name="ps_ss", bufs=1, space="PSUM"))
            ps_tr = ep(tc.tile_pool(name="ps_tr", bufs=1, space="PSUM"))
            dram = ep(tc.tile_pool(name="dram", bufs=1, space="DRAM"))

            # ---------- constants ----------
            ident = constp.tile([128, 128], dt.float32)
            masks.make_identity(nc, ident[:])
            ones_col_bf = constp.tile([128, 1], dt.bfloat16)
            nc.vector.memset(ones_col_bf[:], 1.0)
            lnw_sb = constp.tile([128, IC], dt.float32)    # lnw[128*ic + p] at [p, ic]
            nc.sync.dma_start(lnw_sb[:], lnw_in.rearrange("(c p) -> p c", p=128)[:])
            alnw_sb = constp.tile([128, IC], dt.float32)   # |lnw|
            nc.vector.tensor_scalar(alnw_sb.bitcast(dt.uint32)[:],
                                    lnw_sb.bitcast(dt.uint32)[:],
                                    0x7FFFFFFF, None, AO.bitwise_and)
            # scl columns: [0,0,0, mg, mu, md, 0, 0] broadcast to all partitions
            wstats = constp.tile([128, 8], dt.float32)
            nc.sync.dma_start(wstats[:],
                              scl_in.rearrange("(o f) -> o f", o=1)
                              .partition_broadcast(128))

            # ---------- internal DRAM ----------
            y_partial = dram.tile([T, 2048], dt.bfloat16)
            stat_in = dram.tile([NG, 2, TG], dt.float32)
            stat_out = dram.tile([NG, 2 * N_CORES, TG], dt.float32)
            row_bounce = dram.tile([NG, 2, TG], dt.float32)  # al / cd
            warm_in = dram.tile([8], dt.float32)
            warm_out = dram.tile([8 * N_CORES], dt.float32)

            # tiny first collective: absorbs the one-time CC mesh setup
            # (~80us) while the head DMAs and first matmuls run
            wtile = rowp.tile([1, 8], dt.float32, tag="warm")
            nc.vector.memset(wtile[:], 0.0)
            nc.gpsimd.dma_start(warm_in.rearrange("(o f) -> o f", o=1)[:], wtile[:])
            nc.gpsimd.collective_compute(
                "AllGather", AO.bypass, replica_groups=RG,
                ins=[warm_in.opt()], outs=[warm_out.opt()])

            # ---------- weights: direct fp8 load (order = first-use order) ----
            qwg = wqp.tile([128, KC * ISH], dt.float8e4)
            qwu = wqp.tile([128, KC * ISH], dt.float8e4)
            qwd = wqp.tile([128, IC * 2048], dt.float8e4)

            def emit_wdload():
                for c in range(IC):
                    nc.sync.dma_start(qwd[:, c * 2048:(c + 1) * 2048],
                                      wdT_in[c * 128:(c + 1) * 128, :])

            # ---------- slots ----------
            qxT_slots = {}
            rt_slots = {}
            hT_slots = {}
            h2_slots = {}
            maxt_slots = {}
            cd_slots = {}
            al_slots = {}
            qh_slots = {}

            def emit_load(g):
                tok0 = g * TG
                qxT = qxp.tile([128, KC * TG], dt.bfloat16, tag="qxT")
                qxT_slots[g] = qxT
                for kc in range(KC):
                    nc.sync.dma_start(qxT[:, kc * TG:(kc + 1) * TG],
                                      qxT_in[kc * 128:(kc + 1) * 128,
                                             tok0:tok0 + TG])
                r_tile = sxp.tile([128, TG], dt.float32, tag="r_tile")
                rt_slots[g] = r_tile
                nc.sync.dma_start(r_tile[:], rrow_in[tok0:tok0 + TG]
                                  .rearrange("(o f) -> o f", o=1)
                                  .partition_broadcast(128))

            def emit_phase1(g):
                qxT = qxT_slots.pop(g)
                r_tile = rt_slots.pop(g)
                hT = hbp.tile([128, IC * TG], dt.float32, tag="hT")
                hT_slots[g] = hT
                maxt = smp.tile([128, TG], dt.float32, tag="maxt")
                maxt_slots[g] = maxt
                h2s = []
                h2_slots[g] = h2s
                for ic in range(IC):
                    g_ps = ps_gu.tile([128, TG], dt.float32, tag="gu_ps")
                    u_ps = ps_gu.tile([128, TG], dt.float32, tag="gu_ps")
                    for kc in range(KC):
                        nc.tensor.matmul(
                            g_ps[:],
                            qwg[:, kc * ISH + ic * 128: kc * ISH + (ic + 1) * 128],
                            qxT[:, kc * TG:(kc + 1) * TG],
                            start=(kc == 0), stop=(kc == KC - 1))
                    for kc in range(KC):
                        nc.tensor.matmul(
                            u_ps[:],
                            qwu[:, kc * ISH + ic * 128: kc * ISH + (ic + 1) * 128],
                            qxT[:, kc * TG:(kc + 1) * TG],
                            start=(kc == 0), stop=(kc == KC - 1))
                    gv = evp.tile([128, TG], dt.float32, tag="gv")
                    nc.vector.tensor_tensor(gv[:], g_ps[:], r_tile[:], AO.mult)
                    sv = evp.tile([128, TG], dt.float32, tag="sv")
                    nc.scalar.activation(sv[:], gv[:], AF.Silu,
                                         scale=wstats[:, 3:4])
                    hslice = hT[:, ic * TG:(ic + 1) * TG]
                    nc.vector.scalar_tensor_tensor(hslice, u_ps[:],
                                                   wstats[:, 4:5], sv[:],
                                                   AO.mult, AO.mult)
                    h2 = h2p.tile([128, TG], dt.bfloat16, tag="h2")
                    nc.scalar.activation(h2[:], hslice, AF.Square)
                    h2s.append(h2)
                    if ic == 0:
                        nc.scalar.activation(maxt[:], hslice, AF.Abs,
                                             scale=alnw_sb[:, 0:1])
                    else:
                        ha = evp.tile([128, TG], dt.float32, tag="ha")
                        nc.scalar.activation(ha[:], hslice, AF.Abs,
                                             scale=alnw_sb[:, ic:ic + 1])
                        nc.vector.tensor_tensor(maxt[:], maxt[:], ha[:], AO.max)

            def emit_stats_tail(g):
                # ss matmuls + absmax transposes + stat DMA + AllGather.
                # Emitted after ~33us of down matmuls so all deps are ready.
                h2s = h2_slots.pop(g)
                maxt = maxt_slots.pop(g)
                ss_ps = ps_ss.tile([1, TG], dt.float32, tag="ss_ps")
                for ic in range(IC):
                    nc.tensor.matmul(ss_ps[:], ones_col_bf[:], h2s[ic][:],
                                     start=(ic == 0), stop=(ic == IC - 1))
                ss_row = rowp.tile([1, TG], dt.float32, tag="grow")
                nc.vector.tensor_copy(ss_row[:], ss_ps[:])
                nc.gpsimd.dma_start(stat_in[g, 0].rearrange("(o f) -> o f", o=1)[:],
                                    ss_row[:])
                pm_nat = smp.tile([128, NTC], dt.float32, tag="pm_nat")
                for c in range(NTC):
                    tr_ps = ps_tr.tile([128, 512], dt.float32, tag="tr_ps")
                    nc.tensor.transpose(tr_ps[:, 0:128],
                                        maxt[:, c * 128:(c + 1) * 128], ident[:])
                    nc.vector.tensor_reduce(pm_nat[:, c:c + 1], tr_ps[:, 0:128],
                                            mybir.AxisListType.X, AO.max)
                nc.gpsimd.dma_start(stat_in[g, 1].rearrange("(c p) -> p c", p=128)[:],
                                    pm_nat[:])
                nc.gpsimd.collective_compute(
                    "AllGather", AO.bypass, replica_groups=RG,
                    ins=[stat_in[g].opt()], outs=[stat_out[g].opt()])

            def emit_phase2a(g):
                tok0 = g * TG
                J = TG // 32
                # gathered stats [16, TG] -> [32, TG] tile; rows 16:32 zeroed
                stat32 = smp.tile([32, TG], dt.float32, tag="stat32")
                nc.vector.memset(stat32[:], 0.0)
                nc.gpsimd.dma_start(stat32[0:2 * N_CORES, :], stat_out[g])
                st32 = smp.tile([32, TG], dt.float32, tag="st32")
                nc.vector.transpose(st32[:], stat32[:])
                # st32[q, 32j + 16h + 2a + kind]: token t=32j+q, rank a, h=1 junk
                stv = st32.rearrange("p (j h a two) -> p j h two a",
                                     h=2, two=2, a=N_CORES)
                ssg = smp.tile([32, J], dt.float32, tag="ssg")
                nc.vector.tensor_reduce(ssg[:], stv[:, :, 0:1, 0:1, :],
                                        mybir.AxisListType.X, AO.add)
                pmg = smp.tile([32, J], dt.float32, tag="pmg")
                nc.vector.tensor_reduce(pmg[:], stv[:, :, 0:1, 1:2, :],
                                        mybir.AxisListType.X, AO.max)
                # r residual in [32, J] layout (t = 32j + q)
                r32 = smp.tile([32, J], dt.float32, tag="r32")
                nc.sync.dma_start(r32[:], rrow_in[tok0:tok0 + TG]
                                  .rearrange("(j q) -> q j", q=32)[:])
                nc.vector.tensor_tensor(pmg[:], pmg[:], r32[:], AO.mult)
                rr2 = smp.tile([32, J], dt.float32, tag="rr2")
                nc.vector.tensor_tensor(rr2[:], r32[:], r32[:], AO.mult)
                nc.vector.tensor_tensor(ssg[:], ssg[:], rr2[:], AO.mult)
                vr = smp.tile([32, J], dt.float32, tag="vr")
                nc.vector.tensor_scalar(vr[:], ssg[:], float(1.0 / I), RMS_EPS,
                                        AO.mult, AO.add)
                sq = smp.tile([32, J], dt.float32, tag="sq")
                nc.scalar.sqrt(sq[:], vr[:])
                rr = smp.tile([32, J], dt.float32, tag="rr")
                nc.vector.reciprocal(rr[:], sq[:])
                ntn = smp.tile([32, J], dt.float32, tag="ntn")
                nc.vector.tensor_tensor(ntn[:], sq[:], rr[:], AO.mult)
                nc.vector.tensor_scalar(ntn[:], ntn[:], -1.0, 2.0, AO.mult, AO.add)
                nc.vector.tensor_tensor(rr[:], rr[:], ntn[:], AO.mult)
                rmc = smp.tile([32, J], dt.float32, tag="rmc")
                nc.vector.tensor_tensor(rmc[:], rr[:], pmg[:], AO.mult)
                nc.vector.tensor_scalar(rmc[:], rmc[:], EPS, None, AO.max)
                cd32 = smp.tile([32, J], dt.float32, tag="cd32")
                nc.vector.tensor_scalar(cd32[:], rmc[:], wstats[0:32, 5:6],
                                        float(1.0 / 127.0), AO.mult, AO.mult)
                nc.sync.dma_start(row_bounce[g, 1]
                                  .rearrange("(j q) -> q j", q=32)[:], cd32[:])
                cd = smp.tile([128, NTC], dt.float32, tag="cd")
                cd_slots[g] = cd
                nc.sync.dma_start(cd[:], row_bounce[g, 1]
                                  .rearrange("(c p) -> p c", p=128)[:])
                ar0 = smp.tile([32, J], dt.float32, tag="ar0")
                nc.vector.reciprocal(ar0[:], rmc[:])
                ntn2 = smp.tile([32, J], dt.float32, tag="ntn2")
                nc.vector.tensor_tensor(ntn2[:], rmc[:], ar0[:], AO.mult)
                nc.vector.tensor_scalar(ntn2[:], ntn2[:], -1.0, 2.0, AO.mult, AO.add)
                nc.vector.tensor_tensor(ar0[:], ar0[:], ntn2[:], AO.mult)
                al32 = smp.tile([32, J], dt.float32, tag="al32")
                nc.vector.tensor_tensor(al32[:], rr[:], ar0[:], AO.mult)
                nc.vector.tensor_scalar(al32[:], al32[:], 127.0, None, AO.mult)
                nc.vector.tensor_tensor(al32[:], al32[:], r32[:], AO.mult)
                nc.sync.dma_start(row_bounce[g, 0]
                                  .rearrange("(j q) -> q j", q=32)[:], al32[:])
                al_tile = sxp.tile([128, TG], dt.float32, tag="al_tile")
                al_slots[g] = al_tile
                nc.sync.dma_start(al_tile[:], row_bounce[g, 0]
                                  .rearrange("(o f) -> o f", o=1)
                                  .partition_broadcast(128))

            def emit_phase2q(g):
                hT = hT_slots.pop(g)
                al_tile = al_slots.pop(g)
                # quantize h: round is exact (|h_norm*s| <= 127), clip is dead
                qhT = qhp.tile([128, IC * TG], dt.bfloat16, tag="qhT")
                qh_slots[g] = qhT
                for ic in range(IC):
                    tq = evp.tile([128, TG], dt.float32, tag="hq_t")
                    nc.vector.scalar_tensor_tensor(tq[:], hT[:, ic * TG:(ic + 1) * TG],
                                                   lnw_sb[:, ic:ic + 1], al_tile[:],
                                                   AO.mult, AO.mult)
                    nc.vector.tensor_scalar(qhT[:, ic * TG:(ic + 1) * TG], tq[:],
                                            MAGIC, -MAGIC, AO.add, AO.add)

            def emit_phase2d_tcx(g, tcx):
                tok0 = g * TG
                qhT = qh_slots[g]
                cd = cd_slots[g]
                y_row = yrp.tile([128, 2048], dt.bfloat16, tag="y_row")
                for nh in range(NH):
                    y_ps = ps_dn.tile([128, 512], dt.float32, tag="y_ps")
                    for ic in range(IC):
                        nc.tensor.matmul(
                            y_ps[:],
                            qhT[:, ic * TG + tcx * 128: ic * TG + (tcx + 1) * 128],
                            qwd[:, ic * 2048 + nh * 512: ic * 2048 + (nh + 1) * 512],
                            start=(ic == 0), stop=(ic == IC - 1))
                    nc.scalar.mul(y_row[:, nh * 512:(nh + 1) * 512], y_ps[:],
                                  cd[:, tcx:tcx + 1])
                nc.sync.dma_start(
                    y_partial[tok0 + tcx * 128: tok0 + (tcx + 1) * 128, :],
                    y_row[:])
                if tcx == NTC - 1:
                    qh_slots.pop(g)
                    cd_slots.pop(g)

            TH = TG // 2           # tokens per RS half
            rpb = TH // N_CORES    # 32 output rows per core per half

            rs_outs = dram.tile([NG, 2, rpb, 2048], dt.bfloat16)
            pending_copies = []

            def emit_rs(g, h):
                tok0 = g * TG + h * TH
                nc.gpsimd.collective_compute(
                    "ReduceScatter", AO.add, replica_groups=RG,
                    ins=[y_partial[tok0:tok0 + TH, :].opt()],
                    outs=[rs_outs[g, h].opt()])
                pending_copies.append((g, h))

            def flush_copies(upto_g):
                # y_out copies wait on RS completion; deferring them keeps the
                # gpsimd queue from blocking stat DMAs behind in-flight RS
                for (g, h) in [p for p in pending_copies if p[0] <= upto_g]:
                    r0 = g * 2 * rpb + h * rpb
                    nc.gpsimd.dma_start(y_out[r0:r0 + rpb, :], rs_outs[g, h])
                    pending_copies.remove((g, h))

            # ---------- driver ----------
            # Per iteration g: phase1(g) matmuls; then down-matmuls of g-2 with
            # the stats tail of g (incl. AllGather) inserted after the first
            # token tile so the AG lands on the comms queue ahead of the
            # ReduceScatters and completes a full group before phase2a needs it.
            for kc in range(KC):
                nc.sync.dma_start(qwg[:, kc * ISH:(kc + 1) * ISH],
                                  wgT_in[kc * 128:(kc + 1) * 128, :])
            emit_load(0)
            for kc in range(KC):
                nc.sync.dma_start(qwu[:, kc * ISH:(kc + 1) * ISH],
                                  wuT_in[kc * 128:(kc + 1) * 128, :])
            emit_load(1)
            emit_wdload()
            for g in range(NG):
                if g >= 2:
                    flush_copies(g - 3)
                    emit_phase2d_tcx(g - 2, 0)
                    emit_phase2d_tcx(g - 2, 1)
                    emit_rs(g - 2, 0)
                emit_phase1(g)
                if g >= 2:
                    emit_phase2d_tcx(g - 2, 2)
                    emit_stats_tail(g)
                    emit_phase2d_tcx(g - 2, 3)
                    emit_rs(g - 2, 1)
                else:
                    emit_stats_tail(g)
                if g >= 1:
                    emit_phase2a(g - 1)
                    emit_phase2q(g - 1)
                if g + 2 < NG:
                    emit_load(g + 2)
            # tail: requant of the last group overlaps NG-2's down matmuls
            flush_copies(NG - 3)
            for t in range(2):
                emit_phase2d_tcx(NG - 2, t)
            emit_rs(NG - 2, 0)
            emit_phase2a(NG - 1)
            emit_phase2q(NG - 1)
            for t in range(2, NTC):
                emit_phase2d_tcx(NG - 2, t)
            emit_rs(NG - 2, 1)
            flush_copies(NG - 2)
            for t in range(NTC):
                emit_phase2d_tcx(NG - 1, t)
                if t == 1:
                    emit_rs(NG - 1, 0)
            emit_rs(NG - 1, 1)
            flush_copies(NG - 1)

    nc.compile()
    return nc


def _get_nc():
    if "nc" not in _CACHED:
        _CACHED["nc"] = _build()
    return _CACHED["nc"]


def _host_quant(x, w_gate, w_up, w_down, ln_weight):
    """Replicates reference activation_quant / weight_quant on host."""
    xf = np.asarray(x, dtype=np.float32).reshape(T, H)
    mx = np.clip(np.max(np.abs(xf), axis=1), EPS, None)          # [T]
    sx = np.float32(127.0) / mx.astype(np.float32)
    qx = np.clip(np.rint(xf * sx[:, None]), -128, 127)           # int8 values
    mc = mx.astype(np.float32) / np.float32(127.0)               # dequant scale
    mant, ex = np.frexp(mc)                                      # mc = mant*2^ex
    pow2 = np.ldexp(np.float32(0.5), ex).astype(np.float32)      # 2^(ex-1)
    r = (mant * np.float32(2.0)).astype(np.float32)              # in [1,2)
    qxs = (qx.astype(np.float32) * pow2[:, None])                # exact in bf16
    qxT = np.ascontiguousarray(qxs.T).astype(ml_dtypes.bfloat16)

    def tern(w):
        wf = np.asarray(w, dtype=np.float32)
        m = np.float32(max(np.mean(np.abs(wf), dtype=np.float32), EPS))
        q = np.clip(np.rint(wf * (np.float32(1.0) / m)), -1.0, 1.0)
        return q.astype(ml_dtypes.float8_e4m3), m

    qg, mg = tern(w_gate)    # [I, H]
    qu, mu = tern(w_up)
    qd, md = tern(w_down)    # [H, I]
    scl = np.zeros(8, dtype=np.float32)
    scl[3], scl[4], scl[5] = mg, mu, md
    return qxT, r, qg, qu, qd, scl


def _make_in_maps(x, w_gate, w_up, w_down, ln_weight):
    qxT, r, qg, qu, qd, scl = _host_quant(x, w_gate, w_up, w_down, ln_weight)
    lnw = np.asarray(ln_weight, dtype=np.float32)
    qgT = qg.T    # [H, I] fp8
    quT = qu.T
    qdT = qd.T    # [I, H] fp8
    in_maps = []
    for c in range(N_CORES):
        c0 = c * ISH
        in_maps.append({
            "qxT": qxT,
            "wgT": np.ascontiguousarray(qgT[:, c0:c0 + ISH]),
            "wuT": np.ascontiguousarray(quT[:, c0:c0 + ISH]),
            "wdT": np.ascontiguousarray(qdT[c0:c0 + ISH, :]),
            "lnw": np.ascontiguousarray(lnw[c0:c0 + ISH]),
            "rrow": r,
            "scl": scl,
        })
    return in_maps


def _assemble(results):
    out = np.empty((T, 2048), dtype=np.float32)
    rpb = TG // 2 // N_CORES                           # 32 rows per half
    for c in range(N_CORES):
        yr = np.asarray(results[c]["y_out"]).astype(np.float32)
        for g in range(NG):
            for h in range(2):
                t0 = g * TG + h * (TG // 2) + c * rpb
                r0 = g * 2 * rpb + h * rpb
                out[t0:t0 + rpb] = yr[r0:r0 + rpb]
    return out.reshape(B, S, 2048)


def kernel(x, w_gate, w_up, w_down, ln_weight):
    from concourse import bass_utils

    nc = _get_nc()
    in_maps = _make_in_maps(x, w_gate, w_up, w_down, ln_weight)
    res = bass_utils.run_bass_kernel_spmd(nc, in_maps,
                                          core_ids=list(range(N_CORES)))
    return _assemble(res.results)


# revision 23
# speedup vs baseline: 1.0508x; 1.0508x over previous
"""BitnetMLP on 8 TRN2 NeuronCores — Megatron tensor-parallel over the
intermediate dim I, exact integer arithmetic on the TensorEngine.

v2: all quantization that only needs host-visible data moves to the host:
  - weights are ternarized on host and shipped as fp8e4 {-1,0,+1},
  - x is int8-quantized on host; shipped as bf16 qx*2^e (exact), with the
    per-token residual r = (absmax/127)/2^e in [1,2) shipped as an f32 row.
This removes the on-device weight-stats pass + AllReduce + weight quant pass
and the per-group x-quant prepass entirely, so matmuls start immediately.

Device math per core r (I-shard of 1024):
  g_ps/u_ps = ternary x int8 matmuls (exact, f32 PSUM).
  h/r = silu(g_ps*r*mg)*(u_ps*mu)    (the token residual r folded into stats)
  per-token stats sum(h^2), max|lnw*h| -> AllGather (8 cores) -> rms scale +
  int8 requant scale; qh int8 -> down matmul -> dequant -> bf16 partial ->
  ReduceScatter(add) -> f32 output rows.
"""
import numpy as np
import ml_dtypes

N_CORES = 8
B, S, H, I = 2, 2048, 2048, 8192
T = B * S                      # 4096 tokens
ISH = I // N_CORES             # 1024  I shard per core
TG = 512                       # tokens per group
NG = T // TG                   # 8 groups
KC = H // 128                  # 16 contract chunks for gate/up
IC = ISH // 128                # 8  contract chunks for down / h^T partition chunks
NH = 2048 // 512               # 4  output col groups for down
NTC = TG // 128                # 4  token tiles per group

MAGIC = float(1.5 * 2 ** 23)   # f32 round-to-nearest-even forcing constant
EPS = 1e-5
RMS_EPS = 1e-6

_CACHED = {}


def _build():
    import concourse.bass as bass
    import concourse.bacc as bacc
    import concourse.tile as tile
    import concourse.mybir as mybir
    from concourse import masks
    from contextlib import ExitStack

    dt = mybir.dt
    AO = mybir.AluOpType
    AF = mybir.ActivationFunctionType
    RG = [list(range(N_CORES))]

    nc = bacc.Bacc("TRN2", target_bir_lowering=False, debug=False,
                   num_devices=N_CORES)

    qxT_in = nc.dram_tensor("qxT", [H, T], dt.bfloat16, kind="ExternalInput")
    wgT_in = nc.dram_tensor("wgT", [H, ISH], dt.float8e4, kind="ExternalInput")
    wuT_in = nc.dram_tensor("wuT", [H, ISH], dt.float8e4, kind="ExternalInput")
    wdT_in = nc.dram_tensor("wdT", [ISH, 2048], dt.float8e4,
                            kind="ExternalInput")
    lnw_in = nc.dram_tensor("lnw", [ISH], dt.float32, kind="ExternalInput")
    rrow_in = nc.dram_tensor("rrow", [T], dt.float32, kind="ExternalInput")
    scl_in = nc.dram_tensor("scl", [8], dt.float32, kind="ExternalInput")
    y_out = nc.dram_tensor("y_out", [T // N_CORES, 2048], dt.bfloat16,
                           kind="ExternalOutput")

    with tile.TileContext(nc) as tc:
        with ExitStack() as stack:
            ep = stack.enter_context
            constp = ep(tc.tile_pool(name="const", bufs=1))
            wqp = ep(tc.tile_pool(name="wq", bufs=1))
            qxp = ep(tc.tile_pool(name="qx", bufs=2))
            hbp = ep(tc.tile_pool(name="hbuf", bufs=2))
            qhp = ep(tc.tile_pool(name="qh", bufs=2))
            sxp = ep(tc.tile_pool(name="sxal", bufs=2))
            yrp = ep(tc.tile_pool(name="yrow", bufs=4))
            smp = ep(tc.tile_pool(name="small", bufs=2))
            rowp = ep(tc.tile_pool(name="rows", bufs=2))
            evp = ep(tc.tile_pool(name="evac", bufs=2))
            h2p = ep(tc.tile_pool(name="h2", bufs=10))
            ps_gu = ep(tc.tile_pool(name="ps_gu", bufs=4, space="PSUM"))
            ps_dn = ep(tc.tile_pool(name="ps_dn", bufs=2, space="PSUM"))
            ps_ss = ep(tc.tile_pool(name="ps_ss", bufs=1, space="PSUM"))
            ps_tr = ep(tc.tile_pool(name="ps_tr", bufs=1, space="PSUM"))
            dram = ep(tc.tile_pool(name="dram", bufs=1, space="DRAM"))

            # ---------- constants ----------
            ident = constp.tile([128, 128], dt.float32)
            masks.make_identity(nc, ident[:])
            ones_col_bf = constp.tile([128, 1], dt.bfloat16)
            nc.vector.memset(ones_col_bf[:], 1.0)
            lnw_sb = constp.tile([128, IC], dt.float32)    # lnw[128*ic + p] at [p, ic]
            nc.sync.dma_start(lnw_sb[:], lnw_in.rearrange("(c p) -> p c", p=128)[:])
            alnw_sb = constp.tile([128, IC], dt.float32)   # |lnw|
            nc.vector.tensor_scalar(alnw_sb.bitcast(dt.uint32)[:],
                                    lnw_sb.bitcast(dt.uint32)[:],
                                    0x7FFFFFFF, None, AO.bitwise_and)
            # scl columns: [0,0,0, mg, mu, md, 0, 0] broadcast to all partitions
            wstats = constp.tile([128, 8], dt.float32)
            nc.sync.dma_start(wstats[:],
                              scl_in.rearrange("(o f) -> o f", o=1)
                              .partition_broadcast(128))

            # ---------- internal DRAM ----------
            y_partial = dram.tile([T, 2048], dt.bfloat16)
            stat_in = dram.tile([NG, 2, TG], dt.float32)
            stat_out = dram.tile([NG, 2 * N_CORES, TG], dt.float32)
            row_bounce = dram.tile([NG, 2, TG], dt.float32)  # al / cd
            warm_in = dram.tile([8], dt.float32)
            warm_out = dram.tile([8 * N_CORES], dt.float32)

            # tiny first collective: absorbs the one-time CC mesh setup
            # (~80us) while the head DMAs and first matmuls run
            wtile = rowp.tile([1, 8], dt.float32, tag="warm")
            nc.vector.memset(wtile[:], 0.0)
            nc.gpsimd.dma_start(warm_in.rearrange("(o f) -> o f", o=1)[:], wtile[:])
            nc.gpsimd.collective_compute(
                "AllGather", AO.bypass, replica_groups=RG,
                ins=[warm_in.opt()], outs=[warm_out.opt()])

            # ---------- weights: direct fp8 load (order = first-use order) ----
            qwg = wqp.tile([128, KC * ISH], dt.float8e4)
            qwu = wqp.tile([128, KC * ISH], dt.float8e4)
            qwd = wqp.tile([128, IC * 2048], dt.float8e4)

            def emit_wdload():
                for c in range(IC):
                    nc.sync.dma_start(qwd[:, c * 2048:(c + 1) * 2048],
                                      wdT_in[c * 128:(c + 1) * 128, :])

            # ---------- slots ----------
            qxT_slots = {}
            rt_slots = {}
            hT_slots = {}
            h2_slots = {}
            maxt_slots = {}
            cd_slots = {}
            al_slots = {}
            qh_slots = {}

            def emit_load(g):
                tok0 = g * TG
                qxT = qxp.tile([128, KC * TG], dt.bfloat16, tag="qxT")
                qxT_slots[g] = qxT
                for kc in range(KC):
                    nc.sync.dma_start(qxT[:, kc * TG:(kc + 1) * TG],
                                      qxT_in[kc * 128:(kc + 1) * 128,
                                             tok0:tok0 + TG])
                r_tile = sxp.tile([128, TG], dt.float32, tag="r_tile")
                rt_slots[g] = r_tile
                nc.sync.dma_start(r_tile[:], rrow_in[tok0:tok0 + TG]
                                  .rearrange("(o f) -> o f", o=1)
                                  .partition_broadcast(128))

            def emit_phase1(g):
                qxT = qxT_slots.pop(g)
                r_tile = rt_slots.pop(g)
                hT = hbp.tile([128, IC * TG], dt.float32, tag="hT")
                hT_slots[g] = hT
                maxt = smp.tile([128, TG], dt.float32, tag="maxt")
                maxt_slots[g] = maxt
                h2s = []
                h2_slots[g] = h2s
                for ic in range(IC):
                    g_ps = ps_gu.tile([128, TG], dt.float32, tag="gu_ps")
                    u_ps = ps_gu.tile([128, TG], dt.float32, tag="gu_ps")
                    for kc in range(KC):
                        nc.tensor.matmul(
                            g_ps[:],
                            qwg[:, kc * ISH + ic * 128: kc * ISH + (ic + 1) * 128],
                            qxT[:, kc * TG:(kc + 1) * TG],
                            start=(kc == 0), stop=(kc == KC - 1))
                    for kc in range(KC):
                        nc.tensor.matmul(
                            u_ps[:],
                            qwu[:, kc * ISH + ic * 128: kc * ISH + (ic + 1) * 128],
                            qxT[:, kc * TG:(kc + 1) * TG],
                            start=(kc == 0), stop=(kc == KC - 1))
                    gv = evp.tile([128, TG], dt.float32, tag="gv")
                    nc.vector.tensor_tensor(gv[:], g_ps[:], r_tile[:], AO.mult)
                    sv = evp.tile([128, TG], dt.float32, tag="sv")
                    nc.scalar.activation(sv[:], gv[:], AF.Silu,
                                         scale=wstats[:, 3:4])
                    hslice = hT[:, ic * TG:(ic + 1) * TG]
                    nc.vector.scalar_tensor_tensor(hslice, u_ps[:],
                                                   wstats[:, 4:5], sv[:],
                                                   AO.mult, AO.mult)
                    h2 = h2p.tile([128, TG], dt.bfloat16, tag="h2")
                    nc.scalar.activation(h2[:], hslice, AF.Square)
                    h2s.append(h2)
                    if ic == 0:
                        nc.scalar.activation(maxt[:], hslice, AF.Abs,
                                             scale=alnw_sb[:, 0:1])
                    else:
                        ha = evp.tile([128, TG], dt.float32, tag="ha")
                        nc.scalar.activation(ha[:], hslice, AF.Abs,
                                             scale=alnw_sb[:, ic:ic + 1])
                        nc.vector.tensor_tensor(maxt[:], maxt[:], ha[:], AO.max)

            def emit_stats_tail(g):
                # ss matmuls + absmax transposes + stat DMA + AllGather.
                # Emitted after ~33us of down matmuls so all deps are ready.
                h2s = h2_slots.pop(g)
                maxt = maxt_slots.pop(g)
                ss_ps = ps_ss.tile([1, TG], dt.float32, tag="ss_ps")
                for ic in range(IC):
                    nc.tensor.matmul(ss_ps[:], ones_col_bf[:], h2s[ic][:],
                                     start=(ic == 0), stop=(ic == IC - 1))
                ss_row = rowp.tile([1, TG], dt.float32, tag="grow")
                nc.vector.tensor_copy(ss_row[:], ss_ps[:])
                nc.gpsimd.dma_start(stat_in[g, 0].rearrange("(o f) -> o f", o=1)[:],
                                    ss_row[:])
                pm_nat = smp.tile([128, NTC], dt.float32, tag="pm_nat")
                for c in range(NTC):
                    tr_ps = ps_tr.tile([128, 512], dt.float32, tag="tr_ps")
                    nc.tensor.transpose(tr_ps[:, 0:128],
                                        maxt[:, c * 128:(c + 1) * 128], ident[:])
                    nc.vector.tensor_reduce(pm_nat[:, c:c + 1], tr_ps[:, 0:128],
                                            mybir.AxisListType.X, AO.max)
                nc.gpsimd.dma_start(stat_in[g, 1].rearrange("(c p) -> p c", p=128)[:],
                                    pm_nat[:])
                nc.gpsimd.collective_compute(
                    "AllGather", AO.bypass, replica_groups=RG,
                    ins=[stat_in[g].opt()], outs=[stat_out[g].opt()])

            def emit_phase2a(g):
                tok0 = g * TG
                J = TG // 32
                # gathered stats [16, TG] -> [32, TG] tile; rows 16:32 zeroed
                stat32 = smp.tile([32, TG], dt.float32, tag="stat32")
                nc.vector.memset(stat32[:], 0.0)
                nc.gpsimd.dma_start(stat32[0:2 * N_CORES, :], stat_out[g])
                st32 = smp.tile([32, TG], dt.float32, tag="st32")
                nc.vector.transpose(st32[:], stat32[:])
                # st32[q, 32j + 16h + 2a + kind]: token t=32j+q, rank a, h=1 junk
                stv = st32.rearrange("p (j h a two) -> p j h two a",
                                     h=2, two=2, a=N_CORES)
                ssg = smp.tile([32, J], dt.float32, tag="ssg")
                nc.vector.tensor_reduce(ssg[:], stv[:, :, 0:1, 0:1, :],
                                        mybir.AxisListType.X, AO.add)
                pmg = smp.tile([32, J], dt.float32, tag="pmg")
                nc.vector.tensor_reduce(pmg[:], stv[:, :, 0:1, 1:2, :],
                                        mybir.AxisListType.X, AO.max)
                # r residual in [32, J] layout (t = 32j + q)
                r32 = smp.tile([32, J], dt.float32, tag="r32")
                nc.sync.dma_start(r32[:], rrow_in[tok0:tok0 + TG]
                                  .rearrange("(j q) -> q j", q=32)[:])
                nc.vector.tensor_tensor(pmg[:], pmg[:], r32[:], AO.mult)
                rr2 = smp.tile([32, J], dt.float32, tag="rr2")
                nc.vector.tensor_tensor(rr2[:], r32[:], r32[:], AO.mult)
                nc.vector.tensor_tensor(ssg[:], ssg[:], rr2[:], AO.mult)
                vr = smp.tile([32, J], dt.float32, tag="vr")
                nc.vector.tensor_scalar(vr[:], ssg[:], float(1.0 / I), RMS_EPS,
                                        AO.mult, AO.add)
                sq = smp.tile([32, J], dt.float32, tag="sq")
                nc.scalar.sqrt(sq[:], vr[:])
                rr = smp.tile([32, J], dt.float32, tag="rr")
                nc.vector.reciprocal(rr[:], sq[:])
                ntn = smp.tile([32, J], dt.float32, tag="ntn")
                nc.vector.tensor_tensor(ntn[:], sq[:], rr[:], AO.mult)
                nc.vector.tensor_scalar(ntn[:], ntn[:], -1.0, 2.0, AO.mult, AO.add)
                nc.vector.tensor_tensor(rr[:], rr[:], ntn[:], AO.mult)
                rmc = smp.tile([32, J], dt.float32, tag="rmc")
                nc.vector.tensor_tensor(rmc[:], rr[:], pmg[:], AO.mult)
                nc.vector.tensor_scalar(rmc[:], rmc[:], EPS, None, AO.max)
                cd32 = smp.tile([32, J], dt.float32, tag="cd32")
                nc.vector.tensor_scalar(cd32[:], rmc[:], wstats[0:32, 5:6],
                                        float(1.0 / 127.0), AO.mult, AO.mult)
                nc.sync.dma_start(row_bounce[g, 1]
                                  .rearrange("(j q) -> q j", q=32)[:], cd32[:])
                cd = smp.tile([128, NTC], dt.float32, tag="cd")
                cd_slots[g] = cd
                nc.sync.dma_start(cd[:], row_bounce[g, 1]
                                  .rearrange("(c p) -> p c", p=128)[:])
                ar0 = smp.tile([32, J], dt.float32, tag="ar0")
                nc.vector.reciprocal(ar0[:], rmc[:])
                ntn2 = smp.tile([32, J], dt.float32, tag="ntn2")
                nc.vector.tensor_tensor(ntn2[:], rmc[:], ar0[:], AO.mult)
                nc.vector.tensor_scalar(ntn2[:], ntn2[:], -1.0, 2.0, AO.mult, AO.add)
                nc.vector.tensor_tensor(ar0[:], ar0[:], ntn2[:], AO.mult)
                al32 = smp.tile([32, J], dt.float32, tag="al32")
                nc.vector.tensor_tensor(al32[:], rr[:], ar0[:], AO.mult)
                nc.vector.tensor_scalar(al32[:], al32[:], 127.0, None, AO.mult)
                nc.vector.tensor_tensor(al32[:], al32[:], r32[:], AO.mult)
                nc.sync.dma_start(row_bounce[g, 0]
                                  .rearrange("(j q) -> q j", q=32)[:], al32[:])
                al_tile = sxp.tile([128, TG], dt.float32, tag="al_tile")
                al_slots[g] = al_tile
                nc.sync.dma_start(al_tile[:], row_bounce[g, 0]
                                  .rearrange("(o f) -> o f", o=1)
                                  .partition_broadcast(128))

            def emit_phase2q(g):
                hT = hT_slots.pop(g)
                al_tile = al_slots.pop(g)
                # quantize h: round is exact (|h_norm*s| <= 127), clip is dead
                qhT = qhp.tile([128, IC * TG], dt.bfloat16, tag="qhT")
                qh_slots[g] = qhT
                for ic in range(IC):
                    tq = evp.tile([128, TG], dt.float32, tag="hq_t")
                    nc.vector.scalar_tensor_tensor(tq[:], hT[:, ic * TG:(ic + 1) * TG],
                                                   lnw_sb[:, ic:ic + 1], al_tile[:],
                                                   AO.mult, AO.mult)
                    nc.vector.tensor_scalar(qhT[:, ic * TG:(ic + 1) * TG], tq[:],
                                            MAGIC, -MAGIC, AO.add, AO.add)

            def emit_phase2d_tcx(g, tcx):
                tok0 = g * TG
                qhT = qh_slots[g]
                cd = cd_slots[g]
                y_row = yrp.tile([128, 2048], dt.bfloat16, tag="y_row")
                for nh in range(NH):
                    y_ps = ps_dn.tile([128, 512], dt.float32, tag="y_ps")
                    for ic in range(IC):
                        nc.tensor.matmul(
                            y_ps[:],
                            qhT[:, ic * TG + tcx * 128: ic * TG + (tcx + 1) * 128],
                            qwd[:, ic * 2048 + nh * 512: ic * 2048 + (nh + 1) * 512],
                            start=(ic == 0), stop=(ic == IC - 1))
                    nc.scalar.mul(y_row[:, nh * 512:(nh + 1) * 512], y_ps[:],
                                  cd[:, tcx:tcx + 1])
                nc.sync.dma_start(
                    y_partial[tok0 + tcx * 128: tok0 + (tcx + 1) * 128, :],
                    y_row[:])
                if tcx == NTC - 1:
                    qh_slots.pop(g)
                    cd_slots.pop(g)

            TH = TG // 2           # tokens per RS half
            rpb = TH // N_CORES    # 32 output rows per core per half

            rs_outs = dram.tile([NG, 2, rpb, 2048], dt.bfloat16)
            pending_copies = []

            def emit_rs(g, h):
                tok0 = g * TG + h * TH
                nc.gpsimd.collective_compute(
                    "ReduceScatter", AO.add, replica_groups=RG,
                    ins=[y_partial[tok0:tok0 + TH, :].opt()],
                    outs=[rs_outs[g, h].opt()])
                pending_copies.append((g, h))

            def flush_copies(upto_g):
                # y_out copies wait on RS completion; deferring them keeps the
                # gpsimd queue from blocking stat DMAs behind in-flight RS
                for (g, h) in [p for p in pending_copies if p[0] <= upto_g]:
                    r0 = g * 2 * rpb + h * rpb
                    nc.gpsimd.dma_start(y_out[r0:r0 + rpb, :], rs_outs[g, h])
                    pending_copies.remove((g, h))

            # ---------- driver ----------
            # Per iteration g: phase1(g) matmuls; then down-matmuls of g-2 with
            # the stats tail of g (incl. AllGather) inserted after the first
            # token tile so the AG lands on the comms queue ahead of the
            # ReduceScatters and completes a full group before phase2a needs it.
            for kc in range(KC):
                nc.sync.dma_start(qwg[:, kc * ISH:(kc + 1) * ISH],
                                  wgT_in[kc * 128:(kc + 1) * 128, :])
            emit_load(0)
            for kc in range(KC):
                nc.sync.dma_start(qwu[:, kc * ISH:(kc + 1) * ISH],
                                  wuT_in[kc * 128:(kc + 1) * 128, :])
            emit_load(1)
            emit_wdload()
            for g in range(NG):
                if g >= 3:
                    # down matmuls of g-2 bracket phase1(g): their PSUM-evac
                    # copies never queue behind a full group of SP work
                    flush_copies(g - 3)
                    emit_phase2d_tcx(g - 2, 0)
                    emit_phase2d_tcx(g - 2, 1)
                    emit_rs(g - 2, 0)
                emit_phase1(g)
                if g == 2:
                    # pipe-fill: keep 2d(0) after phase1(2) so the PE isn't
                    # blocked on the first AllGather->requant chain
                    emit_phase2d_tcx(0, 0)
                    emit_phase2d_tcx(0, 1)
                    emit_rs(0, 0)
                if g >= 2:
                    emit_phase2d_tcx(g - 2, 2)
                    emit_stats_tail(g)
                    emit_phase2d_tcx(g - 2, 3)
                    emit_rs(g - 2, 1)
                else:
                    emit_stats_tail(g)
                if g >= 1:
                    emit_phase2a(g - 1)
                    emit_phase2q(g - 1)
                if g + 2 < NG:
                    emit_load(g + 2)
            # tail: requant of the last group overlaps NG-2's down matmuls;
            # all remaining y_out copies go last so they never block the
            # gpsimd queue (they wait on RS completion)
            for t in range(2):
                emit_phase2d_tcx(NG - 2, t)
            emit_rs(NG - 2, 0)
            emit_phase2a(NG - 1)
            emit_phase2q(NG - 1)
            for t in range(2, NTC):
                emit_phase2d_tcx(NG - 2, t)
            emit_rs(NG - 2, 1)
            for t in range(NTC):
                emit_phase2d_tcx(NG - 1, t)
                if t == 1:
                    emit_rs(NG - 1, 0)
            emit_rs(NG - 1, 1)
            flush_copies(NG - 1)

    nc.compile()
    return nc


def _get_nc():
    if "nc" not in _CACHED:
        _CACHED["nc"] = _build()
    return _CACHED["nc"]


def _host_quant(x, w_gate, w_up, w_down, ln_weight):
    """Replicates reference activation_quant / weight_quant on host."""
    xf = np.asarray(x, dtype=np.float32).reshape(T, H)
    mx = np.clip(np.max(np.abs(xf), axis=1), EPS, None)          # [T]
    sx = np.float32(127.0) / mx.astype(np.float32)
    qx = np.clip(np.rint(xf * sx[:, None]), -128, 127)           # int8 values
    mc = mx.astype(np.float32) / np.float32(127.0)               # dequant scale
    mant, ex = np.frexp(mc)                                      # mc = mant*2^ex
    pow2 = np.ldexp(np.float32(0.5), ex).astype(np.float32)      # 2^(ex-1)
    r = (mant * np.float32(2.0)).astype(np.float32)              # in [1,2)
    qxs = (qx.astype(np.float32) * pow2[:, None])                # exact in bf16
    qxT = np.ascontiguousarray(qxs.T).astype(ml_dtypes.bfloat16)

    def tern(w):
        wf = np.asarray(w, dtype=np.float32)
        m = np.float32(max(np.mean(np.abs(wf), dtype=np.float32), EPS))
        q = np.clip(np.rint(wf * (np.float32(1.0) / m)), -1.0, 1.0)
        return q.astype(ml_dtypes.float8_e4m3), m

    qg, mg = tern(w_gate)    # [I, H]
    qu, mu = tern(w_up)
    qd, md = tern(w_down)    # [H, I]
    scl = np.zeros(8, dtype=np.float32)
    scl[3], scl[4], scl[5] = mg, mu, md
    return qxT, r, qg, qu, qd, scl


def _make_in_maps(x, w_gate, w_up, w_down, ln_weight):
    qxT, r, qg, qu, qd, scl = _host_quant(x, w_gate, w_up, w_down, ln_weight)
    lnw = np.asarray(ln_weight, dtype=np.float32)
    qgT = qg.T    # [H, I] fp8
    quT = qu.T
    qdT = qd.T    # [I, H] fp8
    in_maps = []
    for c in range(N_CORES):
        c0 = c * ISH
        in_maps.append({
            "qxT": qxT,
            "wgT": np.ascontiguousarray(qgT[:, c0:c0 + ISH]),
            "wuT": np.ascontiguousarray(quT[:, c0:c0 + ISH]),
            "wdT": np.ascontiguousarray(qdT[c0:c0 + ISH, :]),
            "lnw": np.ascontiguousarray(lnw[c0:c0 + ISH]),
            "rrow": r,
            "scl": scl,
        })
    return in_maps


def _assemble(results):
    out = np.empty((T, 2048), dtype=np.float32)
    rpb = TG // 2 // N_CORES                           # 32 rows per half
    for c in range(N_CORES):
        yr = np.asarray(results[c]["y_out"]).astype(np.float32)
        for g in range(NG):
            for h in range(2):
                t0 = g * TG + h * (TG // 2) + c * rpb
                r0 = g * 2 * rpb + h * rpb
                out[t0:t0 + rpb] = yr[r0:r0 + rpb]
    return out.reshape(B, S, 2048)


def kernel(x, w_gate, w_up, w_down, ln_weight):
    from concourse import bass_utils

    nc = _get_nc()
    in_maps = _make_in_maps(x, w_gate, w_up, w_down, ln_weight)
    res = bass_utils.run_bass_kernel_spmd(nc, in_maps,
                                          core_ids=list(range(N_CORES)))
    return _assemble(res.results)
